# revision 18
# baseline (speedup 1.0000x reference)
"""Trainium2 Bass kernel for nn_Model1 (lag-weighted long-run covariance + MLP).

Math: the 129-lag weighted covariance collapses algebraically:
    sum_l w_l * (Xc @ Y_l.T) = Xc @ (sum_l w_l Y_l).T
so cov*d = Xc @ P.T + N @ Xc.T with P, N two 65-tap causal FIR filters of Xc.
Centering is pushed through the filters as rank-1 corrections so all GEMMs
run on UNCENTERED X:
    cov.T = U.T/d - m (x) alpha - beta (x) m
with U = X@P0.T + N0@X.T (P0,N0 = filters of raw X), m = row means, and
alpha/beta linear in 5 reduction vectors (r,a,c,p,q) that ride along as
extra rows of the U AllReduce.  (m,alpha,beta) = L @ [r;a;c;p;q] for a
constant 5x3 matrix L shipped as a tiny input.

Distribution (8 cores):
  - cov stage: shard time axis (512 cols/core), one bf16 AllReduce of
    [261,256], triggered as early as possible (it doubles as the inter-core
    rendezvous, absorbing launch skew).
  - MLP: tensor-parallel over hidden (512/core).  The activation AllGathers
    between fc1->fc2 and fc2->fc3 are split in 2 chunks each so the second
    chunk's transfer overlaps the first chunk's matmuls.
  - fc3 emits batch-major so proj shards the output columns; final gather is
    a host-side concat.
All heavy GEMMs use bf16 operands with fp32 PSUM accumulation.  All weights
stream during the AllReduce window so post-AR compute is never DMA-gated.
"""
import math
import numpy as np
import ml_dtypes

NCORES = 8
Q = 64
NN = 256          # n (batch/rows of X)
DD = 4096         # d (time axis)
HID = 4096
Y0 = 512
HSH = HID // NCORES    # 512 hidden shard per core
NBLK = DD // 128       # 32 time blocks
BPC = NBLK // NCORES   # 4 blocks per core
KB = HID // 128        # 32 fc2/fc3 contraction blocks

bf16 = ml_dtypes.bfloat16

_CACHE = {}


# ----------------------------------------------------------------------------
# host-side weight-only precompute
# ----------------------------------------------------------------------------
def _erf(x):
    return np.vectorize(math.erf, otypes=[np.float64])(x)


def _gelu64(x):
    return 0.5 * x * (1.0 + _erf(x / np.sqrt(2.0)))


def _filters(inputs):
    f64 = lambda k: np.asarray(inputs[k], np.float64)
    lags = np.arange(-Q, Q + 1, dtype=np.float64)[:, None]
    h = _gelu64(lags @ f64("wn_w1") + f64("wn_b1"))
    w = (h @ f64("wn_w2") + f64("wn_b2"))[:, 0]
    wp = w[Q:]                                    # l = 0..Q
    wnv = np.concatenate([[0.0], w[:Q][::-1]])    # wnv[l] = w[Q-l], l=1..Q
    v = np.arange(128)[:, None]
    u = np.arange(128)[None, :]
    dvu = v - u
    d2 = dvu + 128
    A0 = np.where((dvu >= 0) & (dvu <= Q), wp[np.clip(dvu, 0, Q)], 0.0)
    A1 = np.where((d2 >= 0) & (d2 <= Q), wp[np.clip(d2, 0, Q)], 0.0)
    B0 = np.where((dvu >= 1) & (dvu <= Q), wnv[np.clip(dvu, 0, Q)], 0.0)
    B1 = np.where((d2 >= 1) & (d2 <= Q), wnv[np.clip(d2, 0, Q)], 0.0)
    t = np.arange(DD)
    lim = np.minimum(Q, DD - 1 - t)
    g_p = np.cumsum(wp)[lim]
    g_n = np.cumsum(wnv)[lim]
    gamma = float(g_p.sum() + g_n.sum())
    return A0, A1, B0, B1, g_p, g_n, gamma


# ----------------------------------------------------------------------------
# bass program
# ----------------------------------------------------------------------------
def build(use_gelu=True):
    import concourse.bacc as bacc
    import concourse.tile as tile
    import concourse.mybir as mybir

    dt32 = mybir.dt.float32
    dt16 = mybir.dt.bfloat16
    GELU = (mybir.ActivationFunctionType.Gelu if use_gelu
            else mybir.ActivationFunctionType.Identity)

    nc = bacc.Bacc("TRN2", target_bir_lowering=False, debug=False,
                   num_devices=NCORES)
    mm = nc.tensor.matmul
    RG = [list(range(NCORES))]

    # ---- I/O ----
    xh_d = nc.dram_tensor("xh", [128, 5 * 256], dt16, kind="ExternalInput").ap()
    tp_d = nc.dram_tensor("tp", [128, 512], dt16, kind="ExternalInput").ap()
    aux_d = nc.dram_tensor("aux", [128, BPC * 3], dt16, kind="ExternalInput").ap()
    lm_d = nc.dram_tensor("lm", [5, 4], dt16, kind="ExternalInput").ap()
    xt_d = nc.dram_tensor("xt", [128, NBLK * 256], dt16, kind="ExternalInput").ap()
    w1_d = nc.dram_tensor("w1", [128, 64 * 512], dt16, kind="ExternalInput").ap()
    w2_d = nc.dram_tensor("w2", [128, KB * 512], dt16, kind="ExternalInput").ap()
    w3_d = nc.dram_tensor("w3", [128, KB * 512], dt16, kind="ExternalInput").ap()
    pj_d = nc.dram_tensor("pj", [128, 2 * 512], dt16, kind="ExternalInput").ap()
    b1_d = nc.dram_tensor("b1", [128, 4], dt32, kind="ExternalInput").ap()
    b2_d = nc.dram_tensor("b2", [128, 4], dt32, kind="ExternalInput").ap()
    b3_d = nc.dram_tensor("b3", [1, 512], dt16, kind="ExternalInput").ap()
    out_d = nc.dram_tensor("out", [Y0, HSH], dt16, kind="ExternalOutput").ap()

    with tile.TileContext(nc) as tc:
        with (
            tc.tile_pool(name="cst", bufs=1) as cst,
            tc.tile_pool(name="pn", bufs=3) as pnp,
            tc.tile_pool(name="wst", bufs=3) as wst,
            tc.tile_pool(name="psA", bufs=1, space="PSUM") as psA,
            tc.tile_pool(name="psB", bufs=2, space="PSUM") as psB,
            tc.tile_pool(name="dram", bufs=1, space="DRAM") as drp,
        ):
            # ---------- early small DMAs (stage-1 dependencies first) ----------
            xh_t = cst.tile([128, 5 * 256], dt16, tag="xh")
            nc.sync.dma_start(xh_t, xh_d)
            tp_t = cst.tile([128, 512], dt16, tag="tp")
            nc.scalar.dma_start(tp_t, tp_d)
            aux_t = cst.tile([128, BPC * 3], dt16, tag="aux")
            nc.scalar.dma_start(aux_t, aux_d)
            lm_t = cst.tile([5, 4], dt16, tag="lm")
            nc.scalar.dma_start(lm_t, lm_d)
            b1_t = cst.tile([128, 4], dt32, tag="b1")
            nc.scalar.dma_start(b1_t, b1_d)
            b2_t = cst.tile([128, 4], dt32, tag="b2")
            nc.scalar.dma_start(b2_t, b2_d)
            b3_t = cst.tile([1, 512], dt16, tag="b3")
            nc.scalar.dma_start(b3_t, b3_d)
            ones_t = cst.tile([128, 1], dt16, tag="ones")
            nc.vector.memset(ones_t, 1.0)
            onesr_t = cst.tile([1, 128], dt16, tag="onesr")
            nc.vector.memset(onesr_t, 1.0)

            # ---------- bulk streaming (fills the AllReduce shadow) ----------
            xt_t = cst.tile([128, NBLK * 256], dt16, tag="xt")
            for sp in range(2):
                w = NBLK * 256 // 2
                nc.sync.dma_start(xt_t[:, w * sp: w * (sp + 1)],
                                  xt_d[:, w * sp: w * (sp + 1)])
            # w1 streamed through a rotating pool: 4 G-half + 4 X-half chunks
            wg_tiles = []
            for ch in range(4):
                wt = wst.tile([128, 4096], dt16, tag="w", name=f"wg{ch}")
                nc.sync.dma_start(wt, w1_d[:, 4096 * ch: 4096 * (ch + 1)])
                wg_tiles.append(wt)
            wx_tiles = []
            for ch in range(4):
                wt = wst.tile([128, 4096], dt16, tag="w", name=f"wx{ch}")
                nc.scalar.dma_start(
                    wt, w1_d[:, 16384 + 4096 * ch: 16384 + 4096 * (ch + 1)])
                wx_tiles.append(wt)
            w2R = cst.tile([128, KB * 512], dt16, tag="w2R")
            for sp in range(4):
                w = KB * 512 // 4
                nc.sync.dma_start(w2R[:, w * sp: w * (sp + 1)],
                                  w2_d[:, w * sp: w * (sp + 1)])
            w3R = cst.tile([128, KB * 512], dt16, tag="w3R")
            for sp in range(4):
                w = KB * 512 // 4
                nc.scalar.dma_start(w3R[:, w * sp: w * (sp + 1)],
                                    w3_d[:, w * sp: w * (sp + 1)])
            pj_t = cst.tile([128, 2 * 512], dt16, tag="pj")
            nc.scalar.dma_start(pj_t, pj_d)

            # bounce buffers (DRAM)
            arA_i = drp.tile([261, NN], dt16, tag="arA_i")
            arA_o = drp.tile([261, NN], dt16, tag="arA_o", addr_space="Shared")
            g1_i = [drp.tile([128, 512], dt16, tag=f"g1_i{nh}", name=f"g1_i{nh}")
                    for nh in range(2)]
            g1_o = [drp.tile([1024, 512], dt16, tag=f"g1_o{nh}", name=f"g1_o{nh}",
                             addr_space="Shared") for nh in range(2)]
            g2_i = [drp.tile([128, 512], dt16, tag=f"g2_i{nh}", name=f"g2_i{nh}")
                    for nh in range(2)]
            g2_o = [drp.tile([1024, 512], dt16, tag=f"g2_o{nh}", name=f"g2_o{nh}",
                             addr_space="Shared") for nh in range(2)]

            # ---------- stage 1: cov partials over local time blocks ----------
            # u_ps[ic]: U.T chunk rows [128*ic, 128*ic+128); rac rows (r,a,c);
            # pq row = [p | q] (column sums of the P/N filter outputs)
            u_ps = [psA.tile([128, 256], dt32, tag=f"acc{ic}", name=f"u{ic}")
                    for ic in range(2)]
            rac_ps = psA.tile([3, 256], dt32, tag="acc2", name="rac_ps")
            pq_ps = psA.tile([1, 512], dt32, tag="acc3", name="pq_ps")
            for bl in range(BPC):
                xb = xh_t[:, 256 * bl: 256 * bl + 256]
                xb1 = xh_t[:, 256 * (bl + 1): 256 * (bl + 1) + 256]
                pn = pnp.tile([128, 512], dt16, tag="pn", name="pn")
                pt_ps = psB.tile([128, 256], dt32, tag="rot", name="pt_ps")
                mm(pt_ps, tp_t[:, 0:128], xb, start=True, stop=False)
                mm(pt_ps, tp_t[:, 128:256], xb1, start=False, stop=True)
                nc.vector.tensor_copy(pn[:, 0:256], pt_ps)
                nt_ps = psB.tile([128, 256], dt32, tag="rot", name="nt_ps")
                mm(nt_ps, tp_t[:, 256:384], xb, start=True, stop=False)
                mm(nt_ps, tp_t[:, 384:512], xb1, start=False, stop=True)
                nc.vector.tensor_copy(pn[:, 256:512], nt_ps)
                first, last = bl == 0, bl == BPC - 1
                for ic in range(2):
                    xbc = xh_t[:, 256 * bl + 128 * ic: 256 * bl + 128 * ic + 128]
                    mm(u_ps[ic], pn[:, 128 * ic:128 * ic + 128], xb,
                       start=first, stop=False)
                    mm(u_ps[ic], xbc, pn[:, 256:512], start=False, stop=last)
                mm(rac_ps, aux_t[:, 3 * bl:3 * bl + 3], xb,
                   start=first, stop=last)
                mm(pq_ps, ones_t, pn[:, 0:512], start=first, stop=last)

            # pack (bf16) + stage + AllReduce (doubles as the rendezvous)
            stgs = []
            for ic in range(2):
                stg = cst.tile([128, 256], dt16, tag=f"stg{ic}", name=f"stg{ic}")
                nc.vector.tensor_scalar_mul(stg, u_ps[ic], 1.0 / DD)
                stgs.append(stg)
            vrac = cst.tile([3, 256], dt16, tag="vrac", name="vrac")
            nc.vector.tensor_copy(vrac, rac_ps)
            vpq = cst.tile([1, 512], dt16, tag="vpq", name="vpq")
            nc.vector.tensor_copy(vpq, pq_ps)
            nc.gpsimd.dma_start(arA_i[0:128, :], stgs[0])
            nc.gpsimd.dma_start(arA_i[128:256, :], stgs[1])
            nc.gpsimd.dma_start(arA_i[256:259, :], vrac)
            nc.gpsimd.dma_start(
                arA_i[259:261, :].rearrange("(b p) n -> b p n", p=1)
                .transpose([1, 0, 2]),
                vpq.rearrange("p (b n) -> p b n", b=2))
            nc.gpsimd.collective_compute(
                "AllReduce", mybir.AluOpType.add, replica_groups=RG,
                ins=[arA_i.opt()], outs=[arA_o.opt()])

            # ---------- G^T = X @ W1c (cov-independent, in AR shadow) ----------
            g_ps = [psA.tile([128, 512], dt32, tag=f"acc{4 + ib}", name=f"g_{ib}")
                    for ib in range(2)]
            for ch in range(4):
                for kl in range(8):
                    k = 8 * ch + kl
                    for ib in range(2):
                        mm(g_ps[ib],
                           xt_t[:, 256 * k + 128 * ib: 256 * k + 128 * ib + 128],
                           wg_tiles[ch][:, 512 * kl: 512 * kl + 512],
                           start=(k == 0), stop=(k == NBLK - 1))
            gT = cst.tile([128, 2 * 512], dt16, tag="gT")
            for ib in range(2):
                nc.vector.tensor_copy(gT[:, 512 * ib:512 * ib + 512], g_ps[ib])

            # ---------- fc1 X-half (also in AR shadow) ----------
            f1_ps = [psA.tile([128, 256], dt32, tag=f"acc{hh}", name=f"f1_{hh}")
                     for hh in range(4)]
            for ch in range(4):
                for kl in range(8):
                    k = 8 * ch + kl
                    for hh in range(4):
                        mm(f1_ps[hh],
                           wx_tiles[ch][:, 512 * kl + 128 * hh: 512 * kl + 128 * hh + 128],
                           xt_t[:, 256 * k:256 * k + 256],
                           start=(k == 0), stop=False)

            # ---------- post-AR: m/alpha/beta + cov.T ----------
            rows = cst.tile([5, 256], dt16, tag="rows", name="rows")
            nc.sync.dma_start(rows, arA_o[256:261, :])
            ured = cst.tile([128, 512], dt16, tag="ured", name="ured")
            nc.sync.dma_start(ured[:, 0:256], arA_o[0:128, :])
            nc.scalar.dma_start(ured[:, 256:512], arA_o[128:256, :])
            # m/alpha/beta as three 256-col segments of one partition-0 row
            ma_ps = psB.tile([1, 512], dt32, tag="rot", name="ma_ps")
            for s in range(2):
                mm(ma_ps[0:1, 256 * s: 256 * s + 256], lm_t[:, s:s + 1], rows,
                   start=True, stop=True)
            be_ps = psB.tile([1, 256], dt32, tag="rot", name="be_ps")
            mm(be_ps, lm_t[:, 2:3], rows, start=True, stop=True)
            mab = cst.tile([1, 3 * 256], dt16, tag="mab", name="mab")
            nc.vector.tensor_copy(mab[0:1, 0:512], ma_ps)
            nc.vector.tensor_copy(mab[0:1, 512:768], be_ps)
            covt = cst.tile([128, 2 * 256], dt16, tag="covt")
            for ic in range(2):
                corr = psB.tile([128, 256], dt32, tag="rot", name="corr")
                mm(corr, mab[0:1, 128 * ic:128 * ic + 128], mab[0:1, 256:512],
                   start=True, stop=False)
                mm(corr, mab[0:1, 512 + 128 * ic:512 + 128 * ic + 128],
                   mab[0:1, 0:256], start=False, stop=True)
                nc.vector.tensor_sub(covt[:, 256 * ic:256 * ic + 256],
                                     ured[:, 256 * ic:256 * ic + 256], corr)

            # ---------- fc1 cov contribution + gelu (batch-half pipelined) ----
            # a1loc layout: [128 h, 2 nh * 4 hh * 128 n]  (n-half major);
            # half 0 finishes (stage5 N=128 + gelu) before any half-1 work so
            # its AllGather triggers ~3us earlier.
            a1loc = cst.tile([128, 4 * 256], dt16, tag="a1loc")
            for nh in range(2):
                for hh in range(4):
                    for ib in range(2):
                        mm(f1_ps[hh][:, 128 * nh:128 * nh + 128],
                           gT[:, 512 * ib + 128 * hh: 512 * ib + 128 * hh + 128],
                           covt[:, 256 * ib + 128 * nh:256 * ib + 128 * nh + 128],
                           start=False, stop=(ib == 1 and nh == 1))
                    nc.scalar.activation(
                        a1loc[:, 512 * nh + 128 * hh: 512 * nh + 128 * hh + 128],
                        f1_ps[hh][:, 128 * nh:128 * nh + 128],
                        GELU, bias=b1_t[:, hh:hh + 1])
                nc.scalar.dma_start(g1_i[nh], a1loc[:, 512 * nh:512 * nh + 512])
                nc.gpsimd.collective_compute(
                    "AllGather", mybir.AluOpType.bypass, replica_groups=RG,
                    ins=[g1_i[nh].opt()], outs=[g1_o[nh].opt()])

            # ---------- fc2 (per batch-half, per-rank pipelined loads) ----------
            a1f = [cst.tile([128, 8 * 512], dt16, tag=f"a1f{nh}", name=f"a1f{nh}")
                   for nh in range(2)]
            a2loc = cst.tile([128, 4 * 256], dt16, tag="a2loc")
            f2h = [[psA.tile([128, 128], dt32, tag=f"acc{hh}", name=f"f2_{nh}{hh}")
                    for hh in range(4)] for nh in range(2)]
            for nh in range(2):
                srcg = (g1_o[nh].rearrange("(r p) w -> r p w", p=128)
                        .transpose([1, 0, 2]))
                dst = a1f[nh].rearrange("p (r w) -> p r w", r=8)
                for r in range(8):
                    eng = (nc.sync, nc.scalar, nc.gpsimd)[r % 3]
                    eng.dma_start(dst[:, r:r + 1, :], srcg[:, r:r + 1, :])
                for b in range(KB - 4):
                    r, hh = b // 4, b % 4
                    for h2 in range(4):
                        mm(f2h[nh][h2],
                           w2R[:, 512 * b + 128 * h2: 512 * b + 128 * h2 + 128],
                           a1f[nh][:, 512 * r + 128 * hh: 512 * r + 128 * hh + 128],
                           start=(b == 0), stop=False)
                for h2 in range(4):           # last rank h2-major + inline gelu
                    for b in range(KB - 4, KB):
                        r, hh = b // 4, b % 4
                        mm(f2h[nh][h2],
                           w2R[:, 512 * b + 128 * h2: 512 * b + 128 * h2 + 128],
                           a1f[nh][:, 512 * r + 128 * hh: 512 * r + 128 * hh + 128],
                           start=False, stop=(b == KB - 1))
                    nc.scalar.activation(
                        a2loc[:, 512 * nh + 128 * h2: 512 * nh + 128 * h2 + 128],
                        f2h[nh][h2], GELU, bias=b2_t[:, h2:h2 + 1])
                nc.scalar.dma_start(g2_i[nh], a2loc[:, 512 * nh:512 * nh + 512])
                nc.gpsimd.collective_compute(
                    "AllGather", mybir.AluOpType.bypass, replica_groups=RG,
                    ins=[g2_i[nh].opt()], outs=[g2_o[nh].opt()])

            # ---------- fc3 (batch-major out, per batch-half) ----------
            a2f = [cst.tile([128, 8 * 512], dt16, tag=f"a2f{nh}", name=f"a2f{nh}")
                   for nh in range(2)]
            f3_ps = [psA.tile([128, 512], dt32, tag=f"acc{4 + nh}", name=f"f3_{nh}")
                     for nh in range(2)]
            o3_t = cst.tile([128, 2 * 512], dt16, tag="o3")
            po = [psA.tile([128, 512], dt32, tag=f"acc{pp}", name=f"po{pp}")
                  for pp in range(4)]
            for nh in range(2):
                srcg = (g2_o[nh].rearrange("(r p) w -> r p w", p=128)
                        .transpose([1, 0, 2]))
                dst = a2f[nh].rearrange("p (r w) -> p r w", r=8)
                for r in range(8):
                    eng = (nc.sync, nc.scalar, nc.gpsimd)[r % 3]
                    eng.dma_start(dst[:, r:r + 1, :], srcg[:, r:r + 1, :])
                mm(f3_ps[nh], onesr_t, b3_t, start=True, stop=False)
                for b in range(KB):
                    r, hh = b // 4, b % 4
                    mm(f3_ps[nh],
                       a2f[nh][:, 512 * r + 128 * hh: 512 * r + 128 * hh + 128],
                       w3R[:, 512 * b: 512 * b + 512],
                       start=False, stop=(b == KB - 1))
                # proj contribution of this batch half
                nc.vector.tensor_copy(o3_t[:, 512 * nh:512 * nh + 512], f3_ps[nh])
                for pp in range(4):
                    mm(po[pp],
                       pj_t[:, 512 * nh + 128 * pp: 512 * nh + 128 * pp + 128],
                       o3_t[:, 512 * nh:512 * nh + 512],
                       start=(nh == 0), stop=(nh == 1))
            for pp in range(4):
                osb = cst.tile([128, 512], dt16, tag=f"osb{pp}", name=f"osb{pp}")
                nc.vector.tensor_copy(osb, po[pp])
                nc.sync.dma_start(out_d[128 * pp:128 * pp + 128, :], osb)

    nc.compile()
    return nc


# ----------------------------------------------------------------------------
# host-side sharding / packing
# ----------------------------------------------------------------------------
def prep_in_maps(inputs):
    X = np.asarray(inputs["X"], np.float32)
    A0, A1, B0, B1, g_p, g_n, gamma = _filters(inputs)

    XT = np.ascontiguousarray(X.T)                      # [D, N]
    xt = XT.reshape(NBLK, 128, NN).transpose(1, 0, 2).reshape(128, NBLK * 256)
    xt = xt.astype(bf16)
    tp = np.concatenate([A0, A1, B0, B1], axis=1).astype(bf16)
    pjT = np.asarray(inputs["proj"], np.float64).T      # [256, 512]
    pj = pjT.reshape(2, 128, 512).transpose(1, 0, 2).reshape(128, 1024).astype(bf16)

    lm = np.zeros((5, 4), np.float64)
    lm[0, 0] = 1.0 / DD                 # m  <- r
    lm[0, 1] = -gamma / DD**2           # al <- r
    lm[1, 1] = 1.0 / DD                 # al <- a
    lm[4, 1] = 1.0 / DD                 # al <- q
    lm[2, 2] = 1.0 / DD                 # be <- c
    lm[3, 2] = 1.0 / DD                 # be <- p
    lm = lm.astype(bf16)

    f64 = lambda k: np.asarray(inputs[k], np.float64)
    fc_wT = {1: f64("fc1_w").T, 2: f64("fc2_w").T, 3: f64("fc3_w").T}

    XTz = np.concatenate([XT, np.zeros((128, NN), np.float32)], axis=0)

    in_maps = []
    for c in range(NCORES):
        xh = np.zeros((128, 5 * 256), np.float32)
        for bl in range(5):
            gb = 4 * c + bl
            xh[:, 256 * bl: 256 * bl + 256] = XTz[128 * gb:128 * gb + 128]
        aux = np.zeros((128, BPC * 3), np.float32)
        for bl in range(BPC):
            gb = 4 * c + bl
            aux[:, 3 * bl + 0] = 1.0
            aux[:, 3 * bl + 1] = g_p[128 * gb:128 * gb + 128]
            aux[:, 3 * bl + 2] = g_n[128 * gb:128 * gb + 128]
        hs = slice(HSH * c, HSH * (c + 1))
        # w1: G-half (cov rows 4096..8191) first, then X-half
        w1rows = np.concatenate([fc_wT[1][DD:, hs], fc_wT[1][:DD, hs]], axis=0)
        w1 = w1rows.reshape(64, 128, HSH).transpose(1, 0, 2) \
            .reshape(128, 64 * HSH).astype(bf16)
        w2 = fc_wT[2][:, hs].reshape(KB, 128, HSH) \
            .transpose(1, 0, 2).reshape(128, KB * HSH).astype(bf16)
        w3 = fc_wT[3][:, hs].reshape(KB, 128, HSH) \
            .transpose(1, 0, 2).reshape(128, KB * HSH).astype(bf16)
        b1 = f64("fc1_b")[hs].reshape(4, 128).T.astype(np.float32)
        b2 = f64("fc2_b")[hs].reshape(4, 128).T.astype(np.float32)
        b3 = f64("fc3_b")[hs].reshape(1, HSH).astype(bf16)
        in_maps.append({
            "xt": xt, "xh": xh.astype(bf16), "tp": tp, "lm": lm,
            "aux": aux.astype(bf16), "w1": w1, "w2": w2, "w3": w3,
            "pj": pj, "b1": b1, "b2": b2, "b3": b3,
        })
    return in_maps


def run(inputs, trace=False, **kw):
    in_maps = prep_in_maps(inputs)
    if "nc" not in _CACHE:
        _CACHE["nc"] = build()
    nc = _CACHE["nc"]
    from concourse import bass_utils
    res = bass_utils.run_bass_kernel_spmd(nc, in_maps,
                                          core_ids=list(range(NCORES)),
                                          trace=trace, **kw)
    out = np.concatenate([res.results[c]["out"] for c in range(NCORES)], axis=1)
    return out.astype(np.float32), res


def kernel(**inputs) -> np.ndarray:
    out, _ = run(inputs)
    return out


if __name__ == "__main__":
    data = np.load("inputs.npz")
    inputs = {k: data[k] for k in data.files}
    expected = np.load("expected.npy")
    out = kernel(**inputs)
    scale = np.abs(expected).max()
    err = np.abs(out - expected).max() / scale
    print(f"Relative error: {err:.3e}")
